# revision 1
# baseline (speedup 1.0000x reference)
"""GRU-D kernel for 8 Trainium2 NeuronCores.

Strategy (per sharding hint): data-parallel across batch — axis 0 of
x/delta/m/x_forward is split into 8 shards of 32, weights are replicated
on every core, and the hidden state stays sharded through the whole
scan. Each core runs the full T=256 recurrence for its 32 batch rows;
outputs are gathered back into full-shape arrays on the host.

Implemented as an SPMD jax.pmap program executed on the 8 NeuronCores
(axon PJRT backend). The x/m-conditioned gate inputs (x @ W_*,
(1-m) @ V_*) are hoisted out of the sequential scan as one large batched
matmul over B*T rows, so the per-step recurrence only does the three
h-dependent [32,512]x[512,512] matmuls plus elementwise gate math.
"""

import numpy as np
import jax
import jax.numpy as jnp

B, T, I, H, O = 256, 256, 128, 512, 128
N_CORES = 8
B_LOC = B // N_CORES
BN_EPS = 1e-5

_WEIGHT_NAMES = [
    "W_r", "U_r", "V_r", "b_r",
    "W_z", "U_z", "V_z", "b_z",
    "W", "U", "V", "b",
    "W_gamma_x", "b_gamma_x", "W_gamma_h", "b_gamma_h",
    "dec_W", "dec_b", "bn_gamma", "bn_beta",
]


def _shard_body(x, delta, m, x_forward, w):
    """Runs on one core: full GRU-D for a [B_LOC, T, *] batch shard."""
    # Input decay / imputation (elementwise, hoisted out of the scan)
    decay_x = delta * w["W_gamma_x"] + w["b_gamma_x"]                # [b,T,I]
    x_rep = decay_x * x_forward + (1.0 - decay_x) * 0.001
    x = jnp.where(m > 0, x_rep, x)
    one_minus_m = 1.0 - m

    # Hidden-state decay gamma (batched matmul over all B_LOC*T rows)
    gamma_h = jnp.exp(-jnp.maximum(0.0, m @ w["W_gamma_h"] + w["b_gamma_h"]))

    # Hoist every x/m-dependent matmul out of the scan: precompute the
    # gate pre-activations from x and (1-m) for all timesteps at once.
    pre_z = x @ w["W_z"] + one_minus_m @ w["V_z"] + w["b_z"]         # [b,T,H]
    pre_r = x @ w["W_r"] + one_minus_m @ w["V_r"] + w["b_r"]
    pre_h = x @ w["W"] + one_minus_m @ w["V"] + w["b"]

    # time-major for the scan
    pzs = jnp.transpose(pre_z, (1, 0, 2))
    prs = jnp.transpose(pre_r, (1, 0, 2))
    phs = jnp.transpose(pre_h, (1, 0, 2))
    gs = jnp.transpose(gamma_h, (1, 0, 2))

    U_z, U_r, U = w["U_z"], w["U_r"], w["U"]

    def step(h, inp):
        pz_t, pr_t, ph_t, g_t = inp
        h = g_t * h
        z = jax.nn.sigmoid(pz_t + h @ U_z)
        r = jax.nn.sigmoid(pr_t + h @ U_r)
        h_tilde = jnp.tanh(ph_t + (h * r) @ U)
        h = (1.0 - z) * h + z * h_tilde
        return h, None

    h0 = jnp.zeros((x.shape[0], H), dtype=x.dtype)
    h_t, _ = jax.lax.scan(step, h0, (pzs, prs, phs, gs))

    # BatchNorm1d eval mode (running mean 0 / var 1) + decoder + log_softmax
    h_t = h_t / jnp.sqrt(1.0 + BN_EPS) * w["bn_gamma"] + w["bn_beta"]
    logits = h_t @ w["dec_W"] + w["dec_b"]
    output = jax.nn.log_softmax(logits, axis=-1)
    return output, h_t


_compiled = None


def _get_compiled():
    global _compiled
    if _compiled is None:
        _compiled = jax.pmap(_shard_body, in_axes=(0, 0, 0, 0, None))
    return _compiled


def kernel(**inputs):
    x = np.ascontiguousarray(inputs["x"], dtype=np.float32)
    delta = np.ascontiguousarray(inputs["delta"], dtype=np.float32)
    m = np.ascontiguousarray(inputs["m"], dtype=np.float32)
    x_forward = np.ascontiguousarray(inputs["x_forward"], dtype=np.float32)
    w = {k: np.asarray(inputs[k], dtype=np.float32) for k in _WEIGHT_NAMES}

    # Shard batch across the 8 cores
    xs = x.reshape(N_CORES, B_LOC, T, I)
    ds = delta.reshape(N_CORES, B_LOC, T, I)
    ms = m.reshape(N_CORES, B_LOC, T, I)
    fs = x_forward.reshape(N_CORES, B_LOC, T, I)

    out_sh, h_sh = _get_compiled()(xs, ds, ms, fs, w)

    output = np.asarray(out_sh, dtype=np.float32).reshape(B, O)
    h_t = np.asarray(h_sh, dtype=np.float32).reshape(B, H)
    return output, h_t
